# revision 4
# baseline (speedup 1.0000x reference)
"""AugAttention Trainium2 kernel.

Computes, per batch element (one NeuronCore each, data-parallel over B=8):
  xc = relu(conv1x1(x; Wc, bc))
  q = conv(conv(xc, Wq1), Wq2); k likewise; v likewise
  s = q^T k  (raw scores; softmax/ranking consume s * 1/sqrt(C))
  attn = softmax(s * scale)
  ranks = descending rank of s per row (double argsort)
  mask  = (rank+1)^3 for s >= 0 else 1
  out   = (attn * mask) @ v + xc

Ranking strategy: per row, bucketize s into 8190 buckets over the positive
range (all negatives collapse to bucket 1; masks of negatives don't depend
on their rank), pack = bucket*2048 + column_index (exact in fp32 up to
2^24), bitonic-sort each row's 2048-padded pack vector descending on the
Vector engine, recover the original column of each sorted position from the
low 11 bits, and scatter sorted position t (= rank) back to the original
column with GPSIMD local_scatter.  All matmuls run in fp32 on the PE.
"""
import numpy as np

B, C, H, W = 8, 512, 40, 40
N = H * W            # 1600
NP = 1664            # padded to 13*128
NCH = NP // 128      # 13 chunks of 128 attention rows
NSORT = 2048
SCALE = 1.0 / float(np.sqrt(C))
WNAMES = ["wc", "wq1", "wq2", "wk1", "wk2", "wv1", "wv2"]

_cache = {}


def _sort_stages(n):
    ks = []
    k = 2
    while k <= n:
        j = k // 2
        while j >= 1:
            ks.append((k, j))
            j //= 2
        k *= 2
    return ks


def _build():
    import concourse.bass as bass
    import concourse.mybir as mybir
    import concourse.tile as tile
    from concourse import bacc

    fp32 = mybir.dt.float32
    i32 = mybir.dt.int32
    u16 = mybir.dt.uint16
    i16 = mybir.dt.int16
    A = mybir.AluOpType
    AF = mybir.ActivationFunctionType
    AX = mybir.AxisListType

    nc = bacc.Bacc("TRN2", target_bir_lowering=False, debug=False)

    x_in = nc.declare_dram_parameter("x", [C, NP], fp32, isOutput=False)
    w_in = {n_: nc.declare_dram_parameter(n_, [C, C], fp32, isOutput=False)
            for n_ in WNAMES}  # HOST-TRANSPOSED: w_in[name][c, o] = W[o, c]
    ball_in = nc.declare_dram_parameter("ball", [128, 28], fp32, isOutput=False)
    iou_in = nc.declare_dram_parameter("iotau", [128, NP], u16, isOutput=False)
    id_in = nc.declare_dram_parameter("ident", [128, 128], fp32, isOutput=False)
    out_d = nc.declare_dram_parameter("out", [C, N], fp32, isOutput=True)
    s_dram = nc.dram_tensor("s_scratch", [NCH, 128, NP], fp32)

    with tile.TileContext(nc) as tc:
        with tc.tile_pool(name="sb", bufs=1) as sb, \
             tc.tile_pool(name="wp", bufs=2) as wp, \
             tc.tile_pool(name="sc", bufs=1) as sc, \
             tc.tile_pool(name="ps", bufs=1, space="PSUM") as ps, \
             tc.tile_pool(name="tr", bufs=2, space="PSUM") as trp:

            iota_u = sb.tile([128, NP], u16, tag="iotau")
            nc.sync.dma_start(out=iota_u, in_=iou_in[:, :])
            ident = sb.tile([128, 128], fp32, tag="ident")
            nc.sync.dma_start(out=ident, in_=id_in[:, :])
            ball = sb.tile([128, 28], fp32, tag="ball")
            nc.sync.dma_start(out=ball, in_=ball_in[:, :])

            x_t = []
            for t in range(4):
                xt = sb.tile([128, NP], fp32, tag=f"x{t}")
                nc.sync.dma_start(out=xt, in_=x_in[t * 128:(t + 1) * 128, :])
                x_t.append(xt)

            def conv(src, wname, bias_col, relu, dst_tags):
                wt = []
                for ct in range(4):
                    w = wp.tile([128, C], fp32, tag=f"wt{ct}")
                    nc.sync.dma_start(
                        out=w, in_=w_in[wname][ct * 128:(ct + 1) * 128, :])
                    wt.append(w)
                dst = []
                for ot in range(4):
                    pss = [ps.tile([128, 416], fp32, tag=f"mm{c}", name=f"pss{c}")
                           for c in range(4)]
                    for ct in range(4):
                        lhsT = wt[ct][:, ot * 128:(ot + 1) * 128]
                        for ch in range(4):
                            nc.tensor.matmul(
                                pss[ch], lhsT,
                                src[ct][:, ch * 416:(ch + 1) * 416],
                                start=(ct == 0), stop=(ct == 3))
                    d = sb.tile([128, NP], fp32, tag=dst_tags[ot])
                    for ch in range(4):
                        nc.scalar.activation(
                            out=d[:, ch * 416:(ch + 1) * 416], in_=pss[ch],
                            func=AF.Relu if relu else AF.Identity,
                            bias=ball[:, bias_col + ot:bias_col + ot + 1],
                            scale=1.0)
                    dst.append(d)
                return dst

            xc = conv(x_t, "wc", 0, True, [f"xc{t}" for t in range(4)])
            q1 = conv(xc, "wq1", 4, False, [f"tmp{t}" for t in range(4)])
            q = conv(q1, "wq2", 8, False, [f"q{t}" for t in range(4)])
            k1 = conv(xc, "wk1", 12, False, [f"tmp{t}" for t in range(4)])
            k = conv(k1, "wk2", 16, False, [f"k{t}" for t in range(4)])

            # s chunks: s[nchunk*128 + p, m] = sum_c q[c, n] * k[c, m]
            for i in range(NCH):
                pss = [ps.tile([128, 416], fp32, tag=f"mm{c}", name=f"pss{c}") for c in range(4)]
                for ct in range(4):
                    lhsT = q[ct][:, i * 128:(i + 1) * 128]
                    for ch in range(4):
                        nc.tensor.matmul(
                            pss[ch], lhsT, k[ct][:, ch * 416:(ch + 1) * 416],
                            start=(ct == 0), stop=(ct == 3))
                st = sb.tile([128, NP], fp32, tag="sio", bufs=2, name="st")
                for ch in range(4):
                    nc.scalar.copy(st[:, ch * 416:(ch + 1) * 416], pss[ch])
                nc.vector.memset(st[:, N:NP], -1e6)
                nc.sync.dma_start(out=s_dram[i], in_=st)

            # v after q/k die; reuse their slots
            v1 = conv(xc, "wv1", 20, False, [f"tmp{t}" for t in range(4)])
            v = conv(v1, "wv2", 24, False, [f"k{t}" for t in range(4)])
            vT = []
            for m in range(NCH):
                vt = sb.tile([128, C], fp32, tag=f"vT{m}")
                for ct in range(4):
                    tp = trp.tile([128, 128], fp32, tag="tr")
                    nc.tensor.transpose(tp, v[ct][:, m * 128:(m + 1) * 128], ident)
                    nc.scalar.copy(vt[:, ct * 128:(ct + 1) * 128], tp)
                vT.append(vt)

            stages = _sort_stages(NSORT)

            for i in range(NCH):
                st = sb.tile([128, NP], fp32, tag="sio", bufs=2, name="st")
                nc.sync.dma_start(out=st, in_=s_dram[i])
                mx = sc.tile([128, 1], fp32, tag="mx")
                nc.vector.reduce_max(out=mx, in_=st, axis=AX.X)
                nb = sc.tile([128, 1], fp32, tag="nb")
                nc.vector.tensor_scalar(out=nb, in0=mx, scalar1=-SCALE,
                                        scalar2=None, op0=A.mult)
                e = sb.tile([128, NP], fp32, tag="tmp0")
                z = sc.tile([128, 1], fp32, tag="z")
                nc.scalar.activation(out=e, in_=st, func=AF.Exp, bias=nb,
                                     scale=SCALE, accum_out=z)
                rz = sc.tile([128, 1], fp32, tag="rz")
                nc.vector.reciprocal(out=rz, in_=z)
                pos = sb.tile([128, NP], fp32, tag="tmp1")
                nc.vector.tensor_scalar(out=pos, in0=st, scalar1=0.0,
                                        scalar2=None, op0=A.is_ge)
                mxc = sc.tile([128, 1], fp32, tag="mxc")
                nc.vector.tensor_scalar(out=mxc, in0=mx, scalar1=1e-30,
                                        scalar2=None, op0=A.max)
                rmx = sc.tile([128, 1], fp32, tag="rmx")
                nc.vector.reciprocal(out=rmx, in_=mxc)
                invw = sc.tile([128, 1], fp32, tag="invw")
                nc.vector.tensor_scalar(out=invw, in0=rmx, scalar1=8189.0,
                                        scalar2=None, op0=A.mult)
                tq = sb.tile([128, NP], fp32, tag="tmp2")
                nc.vector.tensor_scalar(out=tq, in0=st, scalar1=invw[:, 0:1],
                                        scalar2=1.5, op0=A.mult, op1=A.add)
                nc.vector.tensor_scalar(out=tq, in0=tq, scalar1=1.0,
                                        scalar2=8191.0, op0=A.max, op1=A.min)
                ci = sb.tile([128, NP], i32, tag="q2")
                nc.vector.tensor_copy(ci, tq)
                pa = sb.tile([128, NSORT], fp32, tag="q0")
                pb = sb.tile([128, NSORT], fp32, tag="q1")
                nc.vector.scalar_tensor_tensor(
                    out=pa[:, :NP], in0=ci, scalar=2048.0, in1=iota_u,
                    op0=A.mult, op1=A.add)
                nc.vector.memset(pa[:, NP:], -1.0)
                cur, oth = pa, pb
                for (kk, jj) in stages:
                    if kk < NSORT:
                        vc = cur.rearrange("p (a d m q r) -> p a d m q r",
                                           d=2, q=2, r=jj,
                                           m=kk // (2 * jj),
                                           a=NSORT // (2 * kk))
                        vo = oth.rearrange("p (a d m q r) -> p a d m q r",
                                           d=2, q=2, r=jj,
                                           m=kk // (2 * jj),
                                           a=NSORT // (2 * kk))
                        nc.vector.tensor_tensor(
                            out=vo[:, :, 0, :, 0, :], in0=vc[:, :, 0, :, 0, :],
                            in1=vc[:, :, 0, :, 1, :], op=A.max)
                        nc.vector.tensor_tensor(
                            out=vo[:, :, 0, :, 1, :], in0=vc[:, :, 0, :, 0, :],
                            in1=vc[:, :, 0, :, 1, :], op=A.min)
                        nc.vector.tensor_tensor(
                            out=vo[:, :, 1, :, 0, :], in0=vc[:, :, 1, :, 0, :],
                            in1=vc[:, :, 1, :, 1, :], op=A.min)
                        nc.vector.tensor_tensor(
                            out=vo[:, :, 1, :, 1, :], in0=vc[:, :, 1, :, 0, :],
                            in1=vc[:, :, 1, :, 1, :], op=A.max)
                    else:
                        vc = cur.rearrange("p (m q r) -> p m q r",
                                           q=2, r=jj, m=NSORT // (2 * jj))
                        vo = oth.rearrange("p (m q r) -> p m q r",
                                           q=2, r=jj, m=NSORT // (2 * jj))
                        nc.vector.tensor_tensor(
                            out=vo[:, :, 0, :], in0=vc[:, :, 0, :],
                            in1=vc[:, :, 1, :], op=A.max)
                        nc.vector.tensor_tensor(
                            out=vo[:, :, 1, :], in0=vc[:, :, 0, :],
                            in1=vc[:, :, 1, :], op=A.min)
                    cur, oth = oth, cur

                ci2 = sb.tile([128, NP], i32, tag="q2")
                nc.vector.tensor_copy(ci2, cur[:, :NP])
                nc.vector.tensor_scalar(out=ci2, in0=ci2, scalar1=2047,
                                        scalar2=None, op0=A.bitwise_and)
                idx16 = sb.tile([128, NP], i16, tag="idx16")
                nc.vector.tensor_copy(idx16, ci2)
                rnk = sb.tile([128, NP], u16, tag="rnk")
                nc.gpsimd.local_scatter(rnk, iota_u, idx16, channels=128,
                                        num_elems=NP, num_idxs=NP)
                r1 = sb.tile([128, NP], fp32, tag="tmp2")
                nc.vector.tensor_scalar(out=r1, in0=rnk, scalar1=1.0,
                                        scalar2=None, op0=A.add)
                r2 = sb.tile([128, NP], fp32, tag="tmp3")
                nc.vector.tensor_tensor(out=r2, in0=r1, in1=r1, op=A.mult)
                nc.vector.tensor_tensor(out=r2, in0=r2, in1=r1, op=A.mult)
                nc.vector.scalar_tensor_tensor(out=r2, in0=r2, scalar=-1.0,
                                               in1=pos, op0=A.add, op1=A.mult)
                nc.vector.tensor_tensor(out=r2, in0=r2, in1=e, op=A.mult)
                nc.vector.tensor_tensor(out=r2, in0=r2, in1=e, op=A.add)
                nc.vector.tensor_scalar(out=st, in0=r2, scalar1=rz[:, 0:1],
                                        scalar2=None, op0=A.mult)

                ats = sb.tile([128, NCH, 128], fp32, tag="q3")
                for m in range(NCH):
                    tp = trp.tile([128, 128], fp32, tag="tr")
                    nc.tensor.transpose(tp, st[:, m * 128:(m + 1) * 128], ident)
                    nc.scalar.copy(ats[:, m, :], tp)
                ncols = 128 if i < NCH - 1 else 64
                for ct in range(4):
                    p4 = ps.tile([128, 128], fp32, tag=f"mm{ct}")
                    for m in range(NCH):
                        nc.tensor.matmul(
                            p4, vT[m][:, ct * 128:(ct + 1) * 128], ats[:, m, :],
                            start=(m == 0), stop=(m == NCH - 1))
                    ob = sb.tile([128, 128], fp32, tag="ob")
                    nc.vector.tensor_tensor(
                        out=ob, in0=p4, in1=xc[ct][:, i * 128:i * 128 + 128],
                        op=A.add)
                    nc.sync.dma_start(
                        out=out_d[ct * 128:(ct + 1) * 128,
                                  i * 128:i * 128 + ncols],
                        in_=ob[:, :ncols])
    nc.compile()
    return nc


def _get_nc():
    if "nc" not in _cache:
        _cache["nc"] = _build()
    return _cache["nc"]


def kernel(x, Wc, bc, Wq1, bq1, Wq2, bq2, Wk1, bk1, Wk2, bk2, Wv1, bv1,
           Wv2, bv2):
    from concourse.bass_utils import run_bass_kernel_spmd

    nc = _get_nc()
    x = np.asarray(x, np.float32)
    ws = {"wc": Wc, "wq1": Wq1, "wq2": Wq2, "wk1": Wk1, "wk2": Wk2,
          "wv1": Wv1, "wv2": Wv2}
    bs = [bc, bq1, bq2, bk1, bk2, bv1, bv2]
    ball = np.zeros((128, 28), np.float32)
    for wi, b in enumerate(bs):
        ball[:, wi * 4:(wi + 1) * 4] = np.asarray(b, np.float32).reshape(4, 128).T
    iotau = np.broadcast_to(np.arange(NP, dtype=np.uint16), (128, NP)).copy()
    ident = np.eye(128, dtype=np.float32)
    wsT = {n_: np.ascontiguousarray(np.asarray(w, np.float32).T)
           for n_, w in ws.items()}

    in_maps = []
    for b_ in range(B):
        xp = np.zeros((C, NP), np.float32)
        xp[:, :N] = x[b_].reshape(C, N)
        m = {"x": xp, "ball": ball, "iotau": iotau, "ident": ident}
        m.update(wsT)
        in_maps.append(m)

    import os
    trace = bool(os.environ.get("KERNEL_TRACE"))
    res = run_bass_kernel_spmd(nc, in_maps, core_ids=list(range(B)),
                               trace=trace)
    kernel._last_results = res
    out = np.stack([res.results[b_]["out"] for b_ in range(B)])
    return out.reshape(B, C, H, W)
